# revision 12
# baseline (speedup 1.0000x reference)
"""Trainium2 Bass kernel for nn_MeanStdStiffRegularizer (segment reduce).

Strategy (8 NeuronCores, segment-bucketed data parallel):
  - The host groups edges by segment id (stable counting sort) and packs
    them into a fixed-capacity padded layout: every segment owns one
    column slot in each of ROUNDS*[128, 512] blocks per core, so column
    index == segment id and partition index == edge slot.  Pad slots
    hold x = 1.0 (log(|1|+eps) ~ 0, square ~ 0 -> pads only bias the
    x-sum by exactly the pad count, which the host subtracts).
  - With position encoding the segment, the device never touches idx:
    per block it computes |x| (DVE bitwise-and on the sign bit, 4x),
    log(|x|+eps) (ACT), log^2 (DVE mult, 2x), then reduces each column
    with a ones-stationary matmul into PSUM (psum column == segment).
    Four PE column tiles each accumulate every 4th block; the 3 value
    streams use 3 PSUM banks.
  - Each core returns [4 tiles, 3 streams, 512 segments] partial sums;
    the host adds tiles/cores, subtracts pad contributions, divides by
    np.bincount counts, and finishes the tiny mean/std loss in float64.
"""

import sys
import types

import numpy as np

N_EDGES = 16777216
NUM_SEG = 512
STRENGTH = 0.01
STD_WEIGHT = 0.5
EPS = 1e-6

N_CORES = 8
P = 128
ROUNDS = 33          # per-core [128, 512] blocks; capacity/segment = 8*33*128
N_PETILE = 4         # PE column tiles (each 32 stationary cols of ones)


def _macro_schedule(rounds):
    """Rounds per macro tile: growing lead-in (ACT starts as soon as the
    first small DMA lands, later DMAs stay ahead), 1-round lead-out
    (short PE/copy/output tail), big middles (low per-op cost)."""
    sched = []
    rem = rounds
    for t in (1, 2, 3, 5, 7):
        if rem - t < 2:
            break
        sched.append(t)
        rem -= t
    while rem > 9:
        sched.append(8)
        rem -= 8
    if rem > 1:
        sched.append(rem - 1)
        rem = 1
    sched.append(rem)
    assert sum(sched) == rounds and sched[-1] == 1
    return sched


def _install_ntff_hook():
    """Register the axon NTFF profiling hook (missing antenv.axon_hooks)."""
    if "antenv.axon_hooks" in sys.modules:
        return
    mod = types.ModuleType("antenv.axon_hooks")
    _h = [None]
    mod.set_axon_ntff_profile_hook = lambda h: _h.__setitem__(0, h)
    mod.get_axon_ntff_profile_hook = lambda: _h[0]
    sys.modules["antenv.axon_hooks"] = mod
    try:
        from trn_agent_boot.trn_boot import _ntff_profile_via_ctypes

        mod.set_axon_ntff_profile_hook(
            _ntff_profile_via_ctypes("/opt/axon/libaxon_pjrt.so")
        )
    except Exception:
        pass


_NO_SPLIT_OPCODES = {
    "CollectiveCompute",
}


def _split_sync_waits(bir_json_bytes):
    """Rewrite BIR so no TPB instruction carries more than one sync wait.

    The walrus codegen in this container supports a single sync-wait slot
    per TPB instruction ("Too many sync wait commands" otherwise).  Extra
    waits are hoisted onto EventSemaphore instructions inserted immediately
    before, on the same engine (same issue-gating semantics).
    """
    import json

    j = json.loads(bir_json_bytes)
    n_split = 0
    uid = [0]
    for f in j["functions"]:
        for b in f["blocks"]:
            out = []
            for ins in b["instructions"]:
                si = ins.get("sync_info")
                ow = (si or {}).get("on_wait") or []
                if len(ow) > 1 and ins.get("opcode") not in _NO_SPLIT_OPCODES:
                    for w in ow[:-1]:
                        uid[0] += 1
                        out.append(
                            {
                                "debug": ins.get("debug", 0),
                                "engine": ins["engine"],
                                "ins": [],
                                "name": f"{ins['name']}-wsplit{uid[0]}",
                                "opcode": "EventSemaphore",
                                "outs": [],
                                "sync_info": {"on_update": [], "on_wait": [w]},
                            }
                        )
                    si["on_wait"] = [ow[-1]]
                    n_split += 1
                out.append(ins)
            b["instructions"] = out
    return json.dumps(j).encode(), n_split


def build_nc(rounds=ROUNDS, n_cores=N_CORES):
    """Build the per-core Bass program (SPMD: same program on every core)."""
    import concourse.bass as bass
    import concourse.tile as tile
    from concourse import mybir

    f32 = mybir.dt.float32
    bf16 = mybir.dt.bfloat16
    i16 = mybir.dt.int16
    AOP = mybir.AluOpType
    ACT = mybir.ActivationFunctionType

    cols = rounds * NUM_SEG
    nc = bass.Bass(
        "TRN2", target_bir_lowering=False, debug=False, num_devices=n_cores
    )
    x_d = nc.dram_tensor("x", [P, cols], bf16, kind="ExternalInput")
    out_d = nc.dram_tensor(
        "out", [N_PETILE, 3, NUM_SEG], f32, kind="ExternalOutput"
    )

    macros = []
    r0 = 0
    for rm in _macro_schedule(rounds):
        macros.append((r0, rm))
        r0 += rm

    with tile.TileContext(nc) as tc:
        with (
            tc.tile_pool(name="const", bufs=1) as cpool,
            tc.tile_pool(name="io", bufs=4) as io,
            tc.tile_pool(name="mid", bufs=2) as mid,
            tc.tile_pool(name="fin", bufs=1) as fin,
            tc.tile_pool(name="acc", bufs=1, space="PSUM") as psum,
        ):
            # warm the 16 DMA engines before the first real input macro
            warm = cpool.tile([P, 64], bf16)
            nc.sync.dma_start(warm[:], x_d[:, 0:64])

            ones = cpool.tile([P, 32], bf16)
            nc.vector.memset(ones[:], 1.0)
            eps_t = cpool.tile([P, 1], f32)
            nc.vector.memset(eps_t[:], EPS)

            # 3 PSUM banks: stream j's per-segment partials; each PE column
            # tile q writes rows [32q, 32q+32) (identical rows: ones cols).
            accs = [
                psum.tile([P, NUM_SEG], f32, tag=f"acc{j}", name=f"acc{j}")
                for j in range(3)
            ]
            # PE col tile for (round, stream): rotate so consecutive MMs
            # hit different array tiles AND different PSUM banks.
            tile_of = lambda r, j: (3 * r + j) % N_PETILE
            n_chain = {}
            for r in range(rounds):
                for j in range(3):
                    k = (j, tile_of(r, j))
                    n_chain[k] = n_chain.get(k, 0) + 1

            mm_done = {k: 0 for k in n_chain}

            def emit_mm(r, j, src, ss):
                q = tile_of(r, j)
                k = (j, q)
                nc.tensor.matmul(
                    accs[j][q * 32 : (q + 1) * 32, :],
                    ones[:, :],
                    src[:, ss],
                    start=(mm_done[k] == 0),
                    stop=(mm_done[k] == n_chain[k] - 1),
                    tile_position=(0, q * 32),
                )
                mm_done[k] += 1

            outsb = fin.tile([P, 3, NUM_SEG], f32)
            for mi, (r0, rm) in enumerate(macros):
                last_macro = mi == len(macros) - 1
                w = rm * NUM_SEG
                cs = slice(r0 * NUM_SEG, r0 * NUM_SEG + w)
                xt = io.tile([P, w], bf16, tag="xt")
                nc.sync.dma_start(xt[:], x_d[:, cs])

                # |x| on DVE: clear the sign bit (single-src int16 -> 4x)
                ax = mid.tile([P, w], bf16, tag="ax")
                nc.vector.tensor_scalar(
                    ax[:].bitcast(i16),
                    xt[:].bitcast(i16),
                    0x7FFF,
                    None,
                    AOP.bitwise_and,
                )
                # log(|x| + eps) on ACT (1x, the bottleneck engine)
                lt = mid.tile([P, w], bf16, tag="lt")
                nc.scalar.activation(lt[:], ax[:], ACT.Ln, bias=eps_t[:])
                # log^2 on DVE (tensor_tensor bf16 -> 2x)
                qt = mid.tile([P, w], bf16, tag="qt")
                nc.vector.tensor_tensor(qt[:], lt[:], lt[:], AOP.mult)

                # stream-major per macro: x MMs gate only on the DMA, l on
                # the Ln, q on the square; in the last macro close each
                # stream's chains and copy its PSUM bank out while the next
                # stream's MMs run.
                for j, src in ((0, xt), (1, lt), (2, qt)):
                    for rr in range(rm):
                        ss = slice(rr * NUM_SEG, (rr + 1) * NUM_SEG)
                        emit_mm(r0 + rr, j, src, ss)
                    if last_macro:
                        if j < 2:
                            nc.scalar.activation(
                                outsb[:, j, :], accs[j][:, :], ACT.Copy
                            )
                        else:
                            nc.vector.tensor_copy(
                                outsb[:, j, :], accs[j][:, :]
                            )
                        nc.sync.dma_start(
                            out_d[:, j, :], outsb[0:P:32, j, :]
                        )

    return nc


_PROG_CACHE = {}


def _get_prog(rounds=ROUNDS):
    if rounds not in _PROG_CACHE:
        nc = build_nc(rounds)
        fixed, _n = _split_sync_waits(nc.to_json_bytes())
        nc.to_json_bytes = lambda: fixed
        _PROG_CACHE[rounds] = nc
    return _PROG_CACHE[rounds]


def _finale(partials, target_mean, target_std):
    """partials: [512, 4] float64 summed across cores -> scalar loss."""
    xs = partials[:, 0]
    ls = partials[:, 1]
    qs = partials[:, 2]
    cnt = partials[:, 3]
    cg = np.maximum(cnt, 1.0)
    mean_w = xs / cg
    mean_log = ls / cg
    log_var = qs / cg - mean_log**2
    std_w = np.sqrt(log_var + EPS)
    mean_loss = np.mean((mean_w - target_mean.astype(np.float64)) ** 2)
    std_loss = np.mean((std_w - target_std.astype(np.float64)) ** 2)
    total = (1.0 - STD_WEIGHT) * mean_loss + STD_WEIGHT * std_loss
    return np.float32(total * STRENGTH)


def _bucketize(x, idx, rounds):
    """Group edges by segment into the padded per-core device layout."""
    import ml_dtypes

    cap = N_CORES * rounds * P
    counts = np.bincount(idx, minlength=NUM_SEG).astype(np.int64)
    order = np.argsort(idx, kind="stable")
    xs = np.asarray(x, dtype=np.float32)[order]
    offs = np.zeros(NUM_SEG + 1, dtype=np.int64)
    np.cumsum(counts, out=offs[1:])

    big = np.full((NUM_SEG, cap), 1.0, dtype=np.float32)
    for s in range(NUM_SEG):
        big[s, : counts[s]] = xs[offs[s] : offs[s + 1]]
    # [seg, core, round, part] -> per core [part, round, seg] flat
    a = big.reshape(NUM_SEG, N_CORES, rounds, P)
    in_maps = []
    for c in range(N_CORES):
        xc = np.ascontiguousarray(a[:, c].transpose(2, 1, 0)).reshape(
            P, rounds * NUM_SEG
        )
        in_maps.append({"x": xc.astype(ml_dtypes.bfloat16)})
    return in_maps, counts


def run_partials(x, idx, trace=False):
    """Run the device program; return [512, 4] partials summed over cores."""
    _install_ntff_hook()
    from concourse.bass_utils import run_bass_kernel_spmd

    x = np.asarray(x, dtype=np.float32)
    idx = np.asarray(idx)

    rounds = ROUNDS
    max_cnt = int(np.bincount(idx, minlength=NUM_SEG).max())
    if max_cnt > N_CORES * rounds * P:  # pathological skew: grow capacity
        rounds = -(-max_cnt // (N_CORES * P)) + 1

    nc = _get_prog(rounds)
    in_maps, counts = _bucketize(x, idx, rounds)
    res = run_bass_kernel_spmd(nc, in_maps, list(range(N_CORES)), trace=trace)

    sums = np.zeros((3, NUM_SEG), dtype=np.float64)
    for c in range(N_CORES):
        o = res.results[c]["out"].astype(np.float64)  # [4, 3, 512]
        sums += o.sum(axis=0)
    pad = N_CORES * rounds * P - counts.astype(np.float64)
    partials = np.zeros((NUM_SEG, 4), dtype=np.float64)
    partials[:, 0] = sums[0] - pad * 1.0          # pads are x = 1.0
    partials[:, 1] = sums[1] - pad * np.log1p(EPS)
    partials[:, 2] = sums[2] - pad * np.log1p(EPS) ** 2
    partials[:, 3] = counts
    return partials, res


def kernel(x, idx, target_mean, target_std):
    partials, _res = run_partials(x, idx, trace=False)
    return _finale(
        partials, np.asarray(target_mean), np.asarray(target_std)
    )


# revision 16
# speedup vs baseline: 1.0141x; 1.0141x over previous
"""Trainium2 Bass kernel for nn_MeanStdStiffRegularizer (segment reduce).

Strategy (8 NeuronCores, segment-bucketed data parallel):
  - The host groups edges by segment id (stable counting sort) and packs
    them into a fixed-capacity padded layout: every segment owns one
    column slot in each of ROUNDS*[128, 512] blocks per core, so column
    index == segment id and partition index == edge slot.  Pad slots
    hold x = 1.0 (log(|1|+eps) ~ 0, square ~ 0 -> pads only bias the
    x-sum by exactly the pad count, which the host subtracts).
  - With position encoding the segment, the device never touches idx:
    per block it computes |x| (DVE bitwise-and on the sign bit, 4x),
    log(|x|+eps) (ACT), log^2 (DVE mult, 2x), then reduces each column
    with a ones-stationary matmul into PSUM (psum column == segment).
    Four PE column tiles each accumulate every 4th block; the 3 value
    streams use 3 PSUM banks.
  - Each core returns [4 tiles, 3 streams, 512 segments] partial sums;
    the host adds tiles/cores, subtracts pad contributions, divides by
    np.bincount counts, and finishes the tiny mean/std loss in float64.
"""

import sys
import types

import numpy as np

N_EDGES = 16777216
NUM_SEG = 512
STRENGTH = 0.01
STD_WEIGHT = 0.5
EPS = 1e-6

N_CORES = 8
P = 128
ROUNDS = 33          # per-core [128, 512] blocks; capacity/segment = 8*33*128
N_PETILE = 4         # PE column tiles (each 32 stationary cols of ones)


def _macro_schedule(rounds):
    """Rounds per macro tile: small lead-in (ACT starts on the first small
    DMA), big middles (low per-op cost), tapered 2+1 lead-out so the final
    serial Ln->square->matmul->copy->DMA chain is as short as possible."""
    sched = [2, 4]
    rem = rounds - 9
    while rem > 8:
        sched.append(8)
        rem -= 8
    sched += [rem, 2, 1]
    assert sum(sched) == rounds and sched[-1] == 1
    return sched


def _install_ntff_hook():
    """Register the axon NTFF profiling hook (missing antenv.axon_hooks)."""
    if "antenv.axon_hooks" in sys.modules:
        return
    mod = types.ModuleType("antenv.axon_hooks")
    _h = [None]
    mod.set_axon_ntff_profile_hook = lambda h: _h.__setitem__(0, h)
    mod.get_axon_ntff_profile_hook = lambda: _h[0]
    sys.modules["antenv.axon_hooks"] = mod
    try:
        from trn_agent_boot.trn_boot import _ntff_profile_via_ctypes

        mod.set_axon_ntff_profile_hook(
            _ntff_profile_via_ctypes("/opt/axon/libaxon_pjrt.so")
        )
    except Exception:
        pass


_NO_SPLIT_OPCODES = {
    "CollectiveCompute",
}


def _split_sync_waits(bir_json_bytes):
    """Rewrite BIR so no TPB instruction carries more than one sync wait.

    The walrus codegen in this container supports a single sync-wait slot
    per TPB instruction ("Too many sync wait commands" otherwise).  Extra
    waits are hoisted onto EventSemaphore instructions inserted immediately
    before, on the same engine (same issue-gating semantics).
    """
    import json

    j = json.loads(bir_json_bytes)
    n_split = 0
    uid = [0]
    for f in j["functions"]:
        for b in f["blocks"]:
            out = []
            for ins in b["instructions"]:
                si = ins.get("sync_info")
                ow = (si or {}).get("on_wait") or []
                if len(ow) > 1 and ins.get("opcode") not in _NO_SPLIT_OPCODES:
                    for w in ow[:-1]:
                        uid[0] += 1
                        out.append(
                            {
                                "debug": ins.get("debug", 0),
                                "engine": ins["engine"],
                                "ins": [],
                                "name": f"{ins['name']}-wsplit{uid[0]}",
                                "opcode": "EventSemaphore",
                                "outs": [],
                                "sync_info": {"on_update": [], "on_wait": [w]},
                            }
                        )
                    si["on_wait"] = [ow[-1]]
                    n_split += 1
                out.append(ins)
            b["instructions"] = out
    return json.dumps(j).encode(), n_split


def build_nc(rounds=ROUNDS, n_cores=N_CORES):
    """Build the per-core Bass program (SPMD: same program on every core)."""
    import concourse.bass as bass
    import concourse.tile as tile
    from concourse import mybir

    f32 = mybir.dt.float32
    bf16 = mybir.dt.bfloat16
    i16 = mybir.dt.int16
    AOP = mybir.AluOpType
    ACT = mybir.ActivationFunctionType

    cols = rounds * NUM_SEG
    nc = bass.Bass(
        "TRN2", target_bir_lowering=False, debug=False, num_devices=n_cores
    )
    x_d = nc.dram_tensor("x", [P, cols], bf16, kind="ExternalInput")
    out_d = nc.dram_tensor(
        "out", [N_PETILE, 3, NUM_SEG], f32, kind="ExternalOutput"
    )

    macros = []
    r0 = 0
    for rm in _macro_schedule(rounds):
        macros.append((r0, rm))
        r0 += rm

    wmax = max(_macro_schedule(rounds)) * NUM_SEG

    with tile.TileContext(nc) as tc:
        with (
            tc.tile_pool(name="const", bufs=1) as cpool,
            tc.tile_pool(name="io", bufs=len(macros)) as io,
            tc.tile_pool(name="mid", bufs=3) as mid,
            tc.tile_pool(name="fin", bufs=1) as fin,
            tc.tile_pool(name="acc", bufs=1, space="PSUM") as psum,
        ):
            # warm the 16 DMA engines before the first real input macro
            warm = cpool.tile([P, 64], bf16)
            nc.sync.dma_start(warm[:], x_d[:, 0:64])

            ones = cpool.tile([P, 32], bf16)
            nc.vector.memset(ones[:], 1.0)
            eps_t = cpool.tile([P, 1], f32)
            nc.vector.memset(eps_t[:], EPS)

            # 3 PSUM banks: stream j's per-segment partials; each PE column
            # tile q writes rows [32q, 32q+32) (identical rows: ones cols).
            accs = [
                psum.tile([P, NUM_SEG], f32, tag=f"acc{j}", name=f"acc{j}")
                for j in range(3)
            ]
            # PE col tile for (round, stream): rotate so consecutive MMs
            # hit different array tiles AND different PSUM banks.
            tile_of = lambda r, j: (3 * r + j) % N_PETILE
            n_chain = {}
            for r in range(rounds):
                for j in range(3):
                    k = (j, tile_of(r, j))
                    n_chain[k] = n_chain.get(k, 0) + 1

            mm_done = {k: 0 for k in n_chain}

            def emit_mm(r, j, src, ss):
                q = tile_of(r, j)
                k = (j, q)
                nc.tensor.matmul(
                    accs[j][q * 32 : (q + 1) * 32, :],
                    ones[:, :],
                    src[:, ss],
                    start=(mm_done[k] == 0),
                    stop=(mm_done[k] == n_chain[k] - 1),
                    tile_position=(0, q * 32),
                )
                mm_done[k] += 1

            outsb = fin.tile([P, 3, NUM_SEG], f32)
            for mi, (r0, rm) in enumerate(macros):
                last_macro = mi == len(macros) - 1
                w = rm * NUM_SEG
                cs = slice(r0 * NUM_SEG, r0 * NUM_SEG + w)
                # uniform-size pool tiles (sliced) so every macro's DMA can
                # be issued up front with no buffer recycling dependency
                xt = io.tile([P, wmax], bf16, tag="xt", name="xt")[:, :w]
                nc.sync.dma_start(xt, x_d[:, cs])

                # |x| on DVE: clear the sign bit (single-src int16 -> 4x)
                ax = mid.tile([P, wmax], bf16, tag="ax", name="ax")[:, :w]
                nc.vector.tensor_scalar(
                    ax.bitcast(i16),
                    xt.bitcast(i16),
                    0x7FFF,
                    None,
                    AOP.bitwise_and,
                )
                # log(|x| + eps) on ACT (1x, the bottleneck engine)
                lt = mid.tile([P, wmax], bf16, tag="lt", name="lt")[:, :w]
                nc.scalar.activation(lt, ax, ACT.Ln, bias=eps_t[:])
                # log^2 on DVE (tensor_tensor bf16 -> 2x)
                qt = mid.tile([P, wmax], bf16, tag="qt", name="qt")[:, :w]
                nc.vector.tensor_tensor(qt, lt, lt, AOP.mult)

                # stream-major per macro: x MMs gate only on the DMA, l on
                # the Ln, q on the square; in the last macro close each
                # stream's chains and copy its PSUM bank out while the next
                # stream's MMs run.
                for j, src in ((0, xt), (1, lt), (2, qt)):
                    for rr in range(rm):
                        ss = slice(rr * NUM_SEG, (rr + 1) * NUM_SEG)
                        emit_mm(r0 + rr, j, src, ss)
                    if last_macro:
                        if j < 2:
                            nc.scalar.activation(
                                outsb[:, j, :], accs[j][:, :], ACT.Copy
                            )
                        else:
                            nc.vector.tensor_copy(
                                outsb[:, j, :], accs[j][:, :]
                            )
            nc.sync.dma_start(out_d[:], outsb[0:P:32, :, :])

    return nc


_PROG_CACHE = {}


def _get_prog(rounds=ROUNDS):
    if rounds not in _PROG_CACHE:
        nc = build_nc(rounds)
        fixed, _n = _split_sync_waits(nc.to_json_bytes())
        nc.to_json_bytes = lambda: fixed
        _PROG_CACHE[rounds] = nc
    return _PROG_CACHE[rounds]


def _finale(partials, target_mean, target_std):
    """partials: [512, 4] float64 summed across cores -> scalar loss."""
    xs = partials[:, 0]
    ls = partials[:, 1]
    qs = partials[:, 2]
    cnt = partials[:, 3]
    cg = np.maximum(cnt, 1.0)
    mean_w = xs / cg
    mean_log = ls / cg
    log_var = qs / cg - mean_log**2
    std_w = np.sqrt(log_var + EPS)
    mean_loss = np.mean((mean_w - target_mean.astype(np.float64)) ** 2)
    std_loss = np.mean((std_w - target_std.astype(np.float64)) ** 2)
    total = (1.0 - STD_WEIGHT) * mean_loss + STD_WEIGHT * std_loss
    return np.float32(total * STRENGTH)


def _bucketize(x, idx, rounds):
    """Group edges by segment into the padded per-core device layout."""
    import ml_dtypes

    cap = N_CORES * rounds * P
    counts = np.bincount(idx, minlength=NUM_SEG).astype(np.int64)
    order = np.argsort(idx, kind="stable")
    xs = np.asarray(x, dtype=np.float32)[order]
    offs = np.zeros(NUM_SEG + 1, dtype=np.int64)
    np.cumsum(counts, out=offs[1:])

    big = np.full((NUM_SEG, cap), 1.0, dtype=np.float32)
    for s in range(NUM_SEG):
        big[s, : counts[s]] = xs[offs[s] : offs[s + 1]]
    # [seg, core, round, part] -> per core [part, round, seg] flat
    a = big.reshape(NUM_SEG, N_CORES, rounds, P)
    in_maps = []
    for c in range(N_CORES):
        xc = np.ascontiguousarray(a[:, c].transpose(2, 1, 0)).reshape(
            P, rounds * NUM_SEG
        )
        in_maps.append({"x": xc.astype(ml_dtypes.bfloat16)})
    return in_maps, counts


def run_partials(x, idx, trace=False):
    """Run the device program; return [512, 4] partials summed over cores."""
    _install_ntff_hook()
    from concourse.bass_utils import run_bass_kernel_spmd

    x = np.asarray(x, dtype=np.float32)
    idx = np.asarray(idx)

    rounds = ROUNDS
    max_cnt = int(np.bincount(idx, minlength=NUM_SEG).max())
    if max_cnt > N_CORES * rounds * P:  # pathological skew: grow capacity
        rounds = -(-max_cnt // (N_CORES * P)) + 1

    nc = _get_prog(rounds)
    in_maps, counts = _bucketize(x, idx, rounds)
    res = run_bass_kernel_spmd(nc, in_maps, list(range(N_CORES)), trace=trace)

    sums = np.zeros((3, NUM_SEG), dtype=np.float64)
    for c in range(N_CORES):
        o = res.results[c]["out"].astype(np.float64)  # [4, 3, 512]
        sums += o.sum(axis=0)
    pad = N_CORES * rounds * P - counts.astype(np.float64)
    partials = np.zeros((NUM_SEG, 4), dtype=np.float64)
    partials[:, 0] = sums[0] - pad * 1.0          # pads are x = 1.0
    partials[:, 1] = sums[1] - pad * np.log1p(EPS)
    partials[:, 2] = sums[2] - pad * np.log1p(EPS) ** 2
    partials[:, 3] = counts
    return partials, res


def kernel(x, idx, target_mean, target_std):
    partials, _res = run_partials(x, idx, trace=False)
    return _finale(
        partials, np.asarray(target_mean), np.asarray(target_std)
    )


# revision 18
# speedup vs baseline: 1.0402x; 1.0258x over previous
"""Trainium2 Bass kernel for nn_MeanStdStiffRegularizer (segment reduce).

Strategy (8 NeuronCores, segment-bucketed data parallel):
  - The host groups edges by segment id (stable counting sort) and packs
    them into a fixed-capacity padded layout: every segment owns one
    column slot in each of ROUNDS*[128, 512] blocks per core, so column
    index == segment id and partition index == edge slot.  Pad slots
    hold x = 1.0 (log(|1|+eps) ~ 0, square ~ 0 -> pads only bias the
    x-sum by exactly the pad count, which the host subtracts).
  - With position encoding the segment, the device never touches idx:
    per block it computes |x| (DVE bitwise-and on the sign bit, 4x),
    log(|x|+eps) (ACT), log^2 (DVE mult, 2x), then reduces each column
    with a ones-stationary matmul into PSUM (psum column == segment).
    Four PE column tiles each accumulate every 4th block; the 3 value
    streams use 3 PSUM banks.
  - Each core returns [4 tiles, 3 streams, 512 segments] partial sums;
    the host adds tiles/cores, subtracts pad contributions, divides by
    np.bincount counts, and finishes the tiny mean/std loss in float64.
"""

import sys
import types

import numpy as np

N_EDGES = 16777216
NUM_SEG = 512
STRENGTH = 0.01
STD_WEIGHT = 0.5
EPS = 1e-6

N_CORES = 8
P = 128
ROUNDS = 33          # per-core [128, 512] blocks; capacity/segment = 8*33*128
N_PETILE = 4         # PE column tiles (each 32 stationary cols of ones)


def _macro_schedule(rounds):
    """Rounds per macro tile: geometric lead-in matched to the ~1.3x
    DMA-vs-ACT rate ratio (ACT starts on the first small DMA and later
    DMAs stay just ahead), big middles (low per-op cost), tapered 4+2+1
    lead-out so each macro's square op hides under the next macro's Ln
    and the final serial square->matmul->copy->DMA chain is short."""
    lead = [1, 2, 3, 5, 7]
    tail = [4, 2, 1]
    mid_total = rounds - sum(lead) - sum(tail)
    assert mid_total >= 0
    sched = list(lead)
    while mid_total > 0:
        take = min(8, mid_total)
        sched.append(take)
        mid_total -= take
    sched += tail
    assert sum(sched) == rounds and sched[-1] == 1
    return sched


def _install_ntff_hook():
    """Register the axon NTFF profiling hook (missing antenv.axon_hooks)."""
    if "antenv.axon_hooks" in sys.modules:
        return
    mod = types.ModuleType("antenv.axon_hooks")
    _h = [None]
    mod.set_axon_ntff_profile_hook = lambda h: _h.__setitem__(0, h)
    mod.get_axon_ntff_profile_hook = lambda: _h[0]
    sys.modules["antenv.axon_hooks"] = mod
    try:
        from trn_agent_boot.trn_boot import _ntff_profile_via_ctypes

        mod.set_axon_ntff_profile_hook(
            _ntff_profile_via_ctypes("/opt/axon/libaxon_pjrt.so")
        )
    except Exception:
        pass


_NO_SPLIT_OPCODES = {
    "CollectiveCompute",
}


def _split_sync_waits(bir_json_bytes):
    """Rewrite BIR so no TPB instruction carries more than one sync wait.

    The walrus codegen in this container supports a single sync-wait slot
    per TPB instruction ("Too many sync wait commands" otherwise).  Extra
    waits are hoisted onto EventSemaphore instructions inserted immediately
    before, on the same engine (same issue-gating semantics).
    """
    import json

    j = json.loads(bir_json_bytes)
    n_split = 0
    uid = [0]
    for f in j["functions"]:
        for b in f["blocks"]:
            out = []
            for ins in b["instructions"]:
                si = ins.get("sync_info")
                ow = (si or {}).get("on_wait") or []
                if len(ow) > 1 and ins.get("opcode") not in _NO_SPLIT_OPCODES:
                    for w in ow[:-1]:
                        uid[0] += 1
                        out.append(
                            {
                                "debug": ins.get("debug", 0),
                                "engine": ins["engine"],
                                "ins": [],
                                "name": f"{ins['name']}-wsplit{uid[0]}",
                                "opcode": "EventSemaphore",
                                "outs": [],
                                "sync_info": {"on_update": [], "on_wait": [w]},
                            }
                        )
                    si["on_wait"] = [ow[-1]]
                    n_split += 1
                out.append(ins)
            b["instructions"] = out
    return json.dumps(j).encode(), n_split


def build_nc(rounds=ROUNDS, n_cores=N_CORES):
    """Build the per-core Bass program (SPMD: same program on every core)."""
    import concourse.bass as bass
    import concourse.tile as tile
    from concourse import mybir

    f32 = mybir.dt.float32
    bf16 = mybir.dt.bfloat16
    i16 = mybir.dt.int16
    AOP = mybir.AluOpType
    ACT = mybir.ActivationFunctionType

    cols = rounds * NUM_SEG
    nc = bass.Bass(
        "TRN2", target_bir_lowering=False, debug=False, num_devices=n_cores
    )
    x_d = nc.dram_tensor("x", [P, cols], bf16, kind="ExternalInput")
    out_d = nc.dram_tensor(
        "out", [N_PETILE, 3, NUM_SEG], f32, kind="ExternalOutput"
    )

    macros = []
    r0 = 0
    for rm in _macro_schedule(rounds):
        macros.append((r0, rm))
        r0 += rm

    wmax = max(_macro_schedule(rounds)) * NUM_SEG

    with tile.TileContext(nc) as tc:
        with (
            tc.tile_pool(name="const", bufs=1) as cpool,
            tc.tile_pool(name="io", bufs=len(macros)) as io,
            tc.tile_pool(name="mid", bufs=3) as mid,
            tc.tile_pool(name="fin", bufs=1) as fin,
            tc.tile_pool(name="acc", bufs=1, space="PSUM") as psum,
        ):
            # warm the 16 DMA engines before the first real input macro
            warm = cpool.tile([P, 64], bf16)
            nc.sync.dma_start(warm[:], x_d[:, 0:64])

            ones = cpool.tile([P, 32], bf16)
            nc.vector.memset(ones[:], 1.0)
            eps_t = cpool.tile([P, 1], f32)
            nc.vector.memset(eps_t[:], EPS)

            # 3 PSUM banks: stream j's per-segment partials; each PE column
            # tile q writes rows [32q, 32q+32) (identical rows: ones cols).
            accs = [
                psum.tile([P, NUM_SEG], f32, tag=f"acc{j}", name=f"acc{j}")
                for j in range(3)
            ]
            # PE col tile for (round, stream): rotate so consecutive MMs
            # hit different array tiles AND different PSUM banks.
            tile_of = lambda r, j: (3 * r + j) % N_PETILE
            n_chain = {}
            for r in range(rounds):
                for j in range(3):
                    k = (j, tile_of(r, j))
                    n_chain[k] = n_chain.get(k, 0) + 1

            mm_done = {k: 0 for k in n_chain}

            def emit_mm(r, j, src, ss):
                q = tile_of(r, j)
                k = (j, q)
                nc.tensor.matmul(
                    accs[j][q * 32 : (q + 1) * 32, :],
                    ones[:, :],
                    src[:, ss],
                    start=(mm_done[k] == 0),
                    stop=(mm_done[k] == n_chain[k] - 1),
                    tile_position=(0, q * 32),
                )
                mm_done[k] += 1

            outsb = fin.tile([P, 3, NUM_SEG], f32)
            for mi, (r0, rm) in enumerate(macros):
                last_macro = mi == len(macros) - 1
                w = rm * NUM_SEG
                cs = slice(r0 * NUM_SEG, r0 * NUM_SEG + w)
                # uniform-size pool tiles (sliced) so every macro's DMA can
                # be issued up front with no buffer recycling dependency
                xt = io.tile([P, wmax], bf16, tag="xt", name="xt")[:, :w]
                nc.sync.dma_start(xt, x_d[:, cs])

                # |x| on DVE: clear the sign bit (single-src int16 -> 4x)
                ax = mid.tile([P, wmax], bf16, tag="ax", name="ax")[:, :w]
                nc.vector.tensor_scalar(
                    ax.bitcast(i16),
                    xt.bitcast(i16),
                    0x7FFF,
                    None,
                    AOP.bitwise_and,
                )
                # log(|x| + eps) on ACT (1x, the bottleneck engine)
                lt = mid.tile([P, wmax], bf16, tag="lt", name="lt")[:, :w]
                nc.scalar.activation(lt, ax, ACT.Ln, bias=eps_t[:])
                # log^2 on DVE (tensor_tensor bf16 -> 2x)
                qt = mid.tile([P, wmax], bf16, tag="qt", name="qt")[:, :w]
                nc.vector.tensor_tensor(qt, lt, lt, AOP.mult)

                # stream-major per macro: x MMs gate only on the DMA, l on
                # the Ln, q on the square; in the last macro close each
                # stream's chains and copy its PSUM bank out while the next
                # stream's MMs run.
                for j, src in ((0, xt), (1, lt), (2, qt)):
                    for rr in range(rm):
                        ss = slice(rr * NUM_SEG, (rr + 1) * NUM_SEG)
                        emit_mm(r0 + rr, j, src, ss)
                    if last_macro:
                        if j < 2:
                            nc.scalar.activation(
                                outsb[:, j, :], accs[j][:, :], ACT.Copy
                            )
                        else:
                            nc.vector.tensor_copy(
                                outsb[:, j, :], accs[j][:, :]
                            )
                        nc.sync.dma_start(
                            out_d[:, j, :], outsb[0:P:32, j, :]
                        )

    return nc


_PROG_CACHE = {}


def _get_prog(rounds=ROUNDS):
    if rounds not in _PROG_CACHE:
        nc = build_nc(rounds)
        fixed, _n = _split_sync_waits(nc.to_json_bytes())
        nc.to_json_bytes = lambda: fixed
        _PROG_CACHE[rounds] = nc
    return _PROG_CACHE[rounds]


def _finale(partials, target_mean, target_std):
    """partials: [512, 4] float64 summed across cores -> scalar loss."""
    xs = partials[:, 0]
    ls = partials[:, 1]
    qs = partials[:, 2]
    cnt = partials[:, 3]
    cg = np.maximum(cnt, 1.0)
    mean_w = xs / cg
    mean_log = ls / cg
    log_var = qs / cg - mean_log**2
    std_w = np.sqrt(log_var + EPS)
    mean_loss = np.mean((mean_w - target_mean.astype(np.float64)) ** 2)
    std_loss = np.mean((std_w - target_std.astype(np.float64)) ** 2)
    total = (1.0 - STD_WEIGHT) * mean_loss + STD_WEIGHT * std_loss
    return np.float32(total * STRENGTH)


def _bucketize(x, idx, rounds):
    """Group edges by segment into the padded per-core device layout."""
    import ml_dtypes

    cap = N_CORES * rounds * P
    counts = np.bincount(idx, minlength=NUM_SEG).astype(np.int64)
    order = np.argsort(idx, kind="stable")
    xs = np.asarray(x, dtype=np.float32)[order]
    offs = np.zeros(NUM_SEG + 1, dtype=np.int64)
    np.cumsum(counts, out=offs[1:])

    big = np.full((NUM_SEG, cap), 1.0, dtype=np.float32)
    for s in range(NUM_SEG):
        big[s, : counts[s]] = xs[offs[s] : offs[s + 1]]
    # [seg, core, round, part] -> per core [part, round, seg] flat
    a = big.reshape(NUM_SEG, N_CORES, rounds, P)
    in_maps = []
    for c in range(N_CORES):
        xc = np.ascontiguousarray(a[:, c].transpose(2, 1, 0)).reshape(
            P, rounds * NUM_SEG
        )
        in_maps.append({"x": xc.astype(ml_dtypes.bfloat16)})
    return in_maps, counts


def run_partials(x, idx, trace=False):
    """Run the device program; return [512, 4] partials summed over cores."""
    _install_ntff_hook()
    from concourse.bass_utils import run_bass_kernel_spmd

    x = np.asarray(x, dtype=np.float32)
    idx = np.asarray(idx)

    rounds = ROUNDS
    max_cnt = int(np.bincount(idx, minlength=NUM_SEG).max())
    if max_cnt > N_CORES * rounds * P:  # pathological skew: grow capacity
        rounds = -(-max_cnt // (N_CORES * P)) + 1

    nc = _get_prog(rounds)
    in_maps, counts = _bucketize(x, idx, rounds)
    res = run_bass_kernel_spmd(nc, in_maps, list(range(N_CORES)), trace=trace)

    sums = np.zeros((3, NUM_SEG), dtype=np.float64)
    for c in range(N_CORES):
        o = res.results[c]["out"].astype(np.float64)  # [4, 3, 512]
        sums += o.sum(axis=0)
    pad = N_CORES * rounds * P - counts.astype(np.float64)
    partials = np.zeros((NUM_SEG, 4), dtype=np.float64)
    partials[:, 0] = sums[0] - pad * 1.0          # pads are x = 1.0
    partials[:, 1] = sums[1] - pad * np.log1p(EPS)
    partials[:, 2] = sums[2] - pad * np.log1p(EPS) ** 2
    partials[:, 3] = counts
    return partials, res


def kernel(x, idx, target_mean, target_std):
    partials, _res = run_partials(x, idx, trace=False)
    return _finale(
        partials, np.asarray(target_mean), np.asarray(target_std)
    )


# revision 20
# speedup vs baseline: 1.1237x; 1.0802x over previous
"""Trainium2 Bass kernel for nn_MeanStdStiffRegularizer (segment reduce).

Strategy (8 NeuronCores, segment-bucketed data parallel):
  - The host groups edges by segment id (stable counting sort) and packs
    them into a fixed-capacity padded layout: every segment owns one
    column slot in each of ROUNDS*[128, 512] blocks per core, so column
    index == segment id and partition index == edge slot.  Pad slots
    hold x = 1.0 (log(|1|+eps) ~ 0, square ~ 0 -> pads only bias the
    x-sum by exactly the pad count, which the host subtracts).
  - x ships as fp8 e5m2 (verified: 1.1e-3 rel err on the final loss,
    vs 2e-2 tolerance; e4m3 fails - its 2^-9 subnormal floor distorts
    log of small |x|).  That halves the input-DMA, which would otherwise
    pace the whole pipeline.
  - With position encoding the segment, the device never touches idx:
    per block it computes |x| (DVE bitwise-and of fp8 PAIRS via an int16
    view, 4x), log(|x|+eps) (ACT, the pacing engine), log^2 (DVE mult,
    2x bf16), then reduces each column with a ones-stationary matmul
    into PSUM (psum column == segment).  The last 1-round macro gets
    host-computed |x| (bf16) and runs entirely on ACT (Ln then Square)
    so the final chain skips the DVE queue.
    Four PE column tiles each accumulate every 4th block; the 3 value
    streams use 3 PSUM banks.
  - Each core returns [4 tiles, 3 streams, 512 segments] partial sums;
    the host adds tiles/cores, subtracts pad contributions, divides by
    np.bincount counts, and finishes the tiny mean/std loss in float64.
"""

import sys
import types

import numpy as np

N_EDGES = 16777216
NUM_SEG = 512
STRENGTH = 0.01
STD_WEIGHT = 0.5
EPS = 1e-6

N_CORES = 8
P = 128
ROUNDS = 33          # per-core [128, 512] blocks; capacity/segment = 8*33*128
N_PETILE = 4         # PE column tiles (each 32 stationary cols of ones)


def _macro_schedule(rounds):
    """Rounds per macro tile: geometric lead-in matched to the ~1.3x
    DMA-vs-ACT rate ratio (ACT starts on the first small DMA and later
    DMAs stay just ahead), big middles (low per-op cost), tapered 4+2+1
    lead-out so each macro's square op hides under the next macro's Ln
    and the final serial square->matmul->copy->DMA chain is short."""
    lead = [1, 2, 3, 5, 7]
    tail = [4, 2, 1]
    mid_total = rounds - sum(lead) - sum(tail)
    assert mid_total >= 0
    sched = list(lead)
    while mid_total > 0:
        take = min(8, mid_total)
        sched.append(take)
        mid_total -= take
    sched += tail
    assert sum(sched) == rounds and sched[-1] == 1
    return sched


def _install_ntff_hook():
    """Register the axon NTFF profiling hook (missing antenv.axon_hooks)."""
    if "antenv.axon_hooks" in sys.modules:
        return
    mod = types.ModuleType("antenv.axon_hooks")
    _h = [None]
    mod.set_axon_ntff_profile_hook = lambda h: _h.__setitem__(0, h)
    mod.get_axon_ntff_profile_hook = lambda: _h[0]
    sys.modules["antenv.axon_hooks"] = mod
    try:
        from trn_agent_boot.trn_boot import _ntff_profile_via_ctypes

        mod.set_axon_ntff_profile_hook(
            _ntff_profile_via_ctypes("/opt/axon/libaxon_pjrt.so")
        )
    except Exception:
        pass


_NO_SPLIT_OPCODES = {
    "CollectiveCompute",
}


def _split_sync_waits(bir_json_bytes):
    """Rewrite BIR so no TPB instruction carries more than one sync wait.

    The walrus codegen in this container supports a single sync-wait slot
    per TPB instruction ("Too many sync wait commands" otherwise).  Extra
    waits are hoisted onto EventSemaphore instructions inserted immediately
    before, on the same engine (same issue-gating semantics).
    """
    import json

    j = json.loads(bir_json_bytes)
    n_split = 0
    uid = [0]
    for f in j["functions"]:
        for b in f["blocks"]:
            out = []
            for ins in b["instructions"]:
                si = ins.get("sync_info")
                ow = (si or {}).get("on_wait") or []
                if len(ow) > 1 and ins.get("opcode") not in _NO_SPLIT_OPCODES:
                    for w in ow[:-1]:
                        uid[0] += 1
                        out.append(
                            {
                                "debug": ins.get("debug", 0),
                                "engine": ins["engine"],
                                "ins": [],
                                "name": f"{ins['name']}-wsplit{uid[0]}",
                                "opcode": "EventSemaphore",
                                "outs": [],
                                "sync_info": {"on_update": [], "on_wait": [w]},
                            }
                        )
                    si["on_wait"] = [ow[-1]]
                    n_split += 1
                out.append(ins)
            b["instructions"] = out
    return json.dumps(j).encode(), n_split


def build_nc(rounds=ROUNDS, n_cores=N_CORES):
    """Build the per-core Bass program (SPMD: same program on every core)."""
    import concourse.bass as bass
    import concourse.tile as tile
    from concourse import mybir

    f32 = mybir.dt.float32
    bf16 = mybir.dt.bfloat16
    i16 = mybir.dt.int16
    AOP = mybir.AluOpType
    ACT = mybir.ActivationFunctionType

    cols = rounds * NUM_SEG
    nc = bass.Bass(
        "TRN2", target_bir_lowering=False, debug=False, num_devices=n_cores
    )
    f8 = mybir.dt.float8e5
    x_d = nc.dram_tensor("x", [P, cols], f8, kind="ExternalInput")
    xa_d = nc.dram_tensor("xa", [P, NUM_SEG], bf16, kind="ExternalInput")
    out_d = nc.dram_tensor(
        "out", [N_PETILE, 3, NUM_SEG], f32, kind="ExternalOutput"
    )

    macros = []
    r0 = 0
    for rm in _macro_schedule(rounds):
        macros.append((r0, rm))
        r0 += rm

    wmax = max(_macro_schedule(rounds)) * NUM_SEG

    with tile.TileContext(nc) as tc:
        with (
            tc.tile_pool(name="const", bufs=1) as cpool,
            tc.tile_pool(name="io", bufs=len(macros)) as io,
            tc.tile_pool(name="mid", bufs=3) as mid,
            tc.tile_pool(name="fin", bufs=1) as fin,
            tc.tile_pool(name="acc", bufs=1, space="PSUM") as psum,
        ):
            # warm the 16 DMA engines before the first real input macro
            warm = cpool.tile([P, 64], f8)
            nc.sync.dma_start(warm[:], x_d[:, 0:64])

            ones = cpool.tile([P, 32], bf16)
            nc.vector.memset(ones[:], 1.0)
            ones8 = cpool.tile([P, 32], f8)
            nc.vector.memset(ones8[:], 1.0)
            eps_t = cpool.tile([P, 1], f32)
            nc.vector.memset(eps_t[:], EPS)

            # 3 PSUM banks: stream j's per-segment partials; each PE column
            # tile q writes rows [32q, 32q+32) (identical rows: ones cols).
            accs = [
                psum.tile([P, NUM_SEG], f32, tag=f"acc{j}", name=f"acc{j}")
                for j in range(3)
            ]
            # PE col tile for (round, stream): rotate so consecutive MMs
            # hit different array tiles AND different PSUM banks.
            tile_of = lambda r, j: (3 * r + j) % N_PETILE
            n_chain = {}
            for r in range(rounds):
                for j in range(3):
                    k = (j, tile_of(r, j))
                    n_chain[k] = n_chain.get(k, 0) + 1

            mm_done = {k: 0 for k in n_chain}

            def emit_mm(r, j, src, ss):
                q = tile_of(r, j)
                k = (j, q)
                nc.tensor.matmul(
                    accs[j][q * 32 : (q + 1) * 32, :],
                    (ones8 if j == 0 else ones)[:, :],
                    src[:, ss],
                    start=(mm_done[k] == 0),
                    stop=(mm_done[k] == n_chain[k] - 1),
                    tile_position=(0, q * 32),
                )
                mm_done[k] += 1

            outsb = fin.tile([P, 3, NUM_SEG], f32)
            for mi, (r0, rm) in enumerate(macros):
                last_macro = mi == len(macros) - 1
                w = rm * NUM_SEG
                cs = slice(r0 * NUM_SEG, r0 * NUM_SEG + w)
                # uniform-size pool tiles (sliced) so every macro's DMA can
                # be issued up front with no buffer recycling dependency
                xt = io.tile([P, wmax], f8, tag="xt", name="xt")[:, :w]
                nc.sync.dma_start(xt, x_d[:, cs])

                lt = mid.tile([P, wmax], bf16, tag="lt", name="lt")[:, :w]
                qt = mid.tile([P, wmax], bf16, tag="qt", name="qt")[:, :w]
                if not last_macro:
                    # |x| on DVE: clear the fp8 sign bits two at a time
                    # (single-src int16 view, step-1 -> 4x)
                    ax = mid.tile([P, wmax], f8, tag="ax", name="ax")[:, :w]
                    nc.vector.tensor_scalar(
                        ax.bitcast(i16),
                        xt.bitcast(i16),
                        0x7F7F,
                        None,
                        AOP.bitwise_and,
                    )
                    # log(|x| + eps) on ACT (1x, the pacing engine)
                    nc.scalar.activation(lt, ax, ACT.Ln, bias=eps_t[:])
                    # log^2 on DVE (tensor_tensor bf16 -> 2x)
                    nc.vector.tensor_tensor(qt, lt, lt, AOP.mult)
                else:
                    # final round: host-supplied |x| (bf16), ACT-only
                    # chain so the tail skips the DVE queue entirely
                    xa = io.tile([P, NUM_SEG], bf16, tag="xa", name="xa")
                    nc.sync.dma_start(xa[:], xa_d[:, :])
                    nc.scalar.activation(lt, xa[:], ACT.Ln, bias=eps_t[:])
                    nc.scalar.activation(qt, lt, ACT.Square)

                # stream-major per macro: x MMs gate only on the DMA, l on
                # the Ln, q on the square; in the last macro close each
                # stream's chains and copy its PSUM bank out while the next
                # stream's MMs run.
                for j, src in ((0, xt), (1, lt), (2, qt)):
                    for rr in range(rm):
                        ss = slice(rr * NUM_SEG, (rr + 1) * NUM_SEG)
                        emit_mm(r0 + rr, j, src, ss)
                    if last_macro:
                        if j < 2:
                            nc.scalar.activation(
                                outsb[:, j, :], accs[j][:, :], ACT.Copy
                            )
                        else:
                            nc.vector.tensor_copy(
                                outsb[:, j, :], accs[j][:, :]
                            )
                        nc.sync.dma_start(
                            out_d[:, j, :], outsb[0:P:32, j, :]
                        )

    return nc


_PROG_CACHE = {}


def _get_prog(rounds=ROUNDS):
    if rounds not in _PROG_CACHE:
        nc = build_nc(rounds)
        fixed, _n = _split_sync_waits(nc.to_json_bytes())
        nc.to_json_bytes = lambda: fixed
        _PROG_CACHE[rounds] = nc
    return _PROG_CACHE[rounds]


def _finale(partials, target_mean, target_std):
    """partials: [512, 4] float64 summed across cores -> scalar loss."""
    xs = partials[:, 0]
    ls = partials[:, 1]
    qs = partials[:, 2]
    cnt = partials[:, 3]
    cg = np.maximum(cnt, 1.0)
    mean_w = xs / cg
    mean_log = ls / cg
    log_var = qs / cg - mean_log**2
    std_w = np.sqrt(log_var + EPS)
    mean_loss = np.mean((mean_w - target_mean.astype(np.float64)) ** 2)
    std_loss = np.mean((std_w - target_std.astype(np.float64)) ** 2)
    total = (1.0 - STD_WEIGHT) * mean_loss + STD_WEIGHT * std_loss
    return np.float32(total * STRENGTH)


def _bucketize(x, idx, rounds):
    """Group edges by segment into the padded per-core device layout."""
    import ml_dtypes

    cap = N_CORES * rounds * P
    counts = np.bincount(idx, minlength=NUM_SEG).astype(np.int64)
    order = np.argsort(idx, kind="stable")
    xs = np.asarray(x, dtype=np.float32)[order]
    offs = np.zeros(NUM_SEG + 1, dtype=np.int64)
    np.cumsum(counts, out=offs[1:])

    big = np.full((NUM_SEG, cap), 1.0, dtype=np.float32)
    for s in range(NUM_SEG):
        big[s, : counts[s]] = xs[offs[s] : offs[s + 1]]
    # [seg, core, round, part] -> per core [part, round, seg] flat
    a = big.reshape(NUM_SEG, N_CORES, rounds, P)
    in_maps = []
    for c in range(N_CORES):
        xc = np.ascontiguousarray(a[:, c].transpose(2, 1, 0)).reshape(
            P, rounds * NUM_SEG
        )
        x8 = xc.astype(ml_dtypes.float8_e5m2)
        xa = np.abs(
            x8[:, -NUM_SEG:].astype(np.float32)
        ).astype(ml_dtypes.bfloat16)
        in_maps.append({"x": x8, "xa": xa})
    return in_maps, counts


def run_partials(x, idx, trace=False):
    """Run the device program; return [512, 4] partials summed over cores."""
    _install_ntff_hook()
    from concourse.bass_utils import run_bass_kernel_spmd

    x = np.asarray(x, dtype=np.float32)
    idx = np.asarray(idx)

    rounds = ROUNDS
    max_cnt = int(np.bincount(idx, minlength=NUM_SEG).max())
    if max_cnt > N_CORES * rounds * P:  # pathological skew: grow capacity
        rounds = -(-max_cnt // (N_CORES * P)) + 1

    nc = _get_prog(rounds)
    in_maps, counts = _bucketize(x, idx, rounds)
    res = run_bass_kernel_spmd(nc, in_maps, list(range(N_CORES)), trace=trace)

    sums = np.zeros((3, NUM_SEG), dtype=np.float64)
    for c in range(N_CORES):
        o = res.results[c]["out"].astype(np.float64)  # [4, 3, 512]
        sums += o.sum(axis=0)
    pad = N_CORES * rounds * P - counts.astype(np.float64)
    partials = np.zeros((NUM_SEG, 4), dtype=np.float64)
    partials[:, 0] = sums[0] - pad * 1.0          # pads are x = 1.0
    partials[:, 1] = sums[1] - pad * np.log1p(EPS)
    partials[:, 2] = sums[2] - pad * np.log1p(EPS) ** 2
    partials[:, 3] = counts
    return partials, res


def kernel(x, idx, target_mean, target_std):
    partials, _res = run_partials(x, idx, trace=False)
    return _finale(
        partials, np.asarray(target_mean), np.asarray(target_std)
    )
